# revision 1
# baseline (speedup 1.0000x reference)
"""CRD (contrastive representation distillation) loss on 8 Trainium2 cores.

Strategy (memory-bound scatter-gather problem):
  - The two 500000x128 memory banks are sharded row-wise across the 8 cores
    (62500 rows each, split in two 31250-row halves so gather indices fit in
    int16). Banks are converted to bf16 on the host.
  - The host computes, per core, the list of sampled (b, k) pairs whose memory
    row falls in that core's shard, sorts them by batch index b, pads each
    b-run to a multiple of 128, and ships the local row indices as int16
    dma_gather index tensors.
  - On device, dma_gather(transpose=True) pulls the sampled rows from HBM
    *already transposed* into SBUF tiles [d=128 partitions, pairs]. Each
    128-pair chunk is used as the stationary matmul operand (lhsT) against a
    single embedding column (rhs = e_b^T), so the 128 dot products of a chunk
    land spread across the 128 PSUM partitions: no scalar extraction needed.
  - Embeddings es/et (linear head + L2 norm) are computed on every core
    (replicated, trivial). Per-chunk rhs columns are produced with a second
    tiny dma_gather from the embedding matrix using the per-chunk b list.
  - The loss reduces to four masked sums over the packed pair scores
    (sum exp(s/T) for Z, sum log1p-style terms, positive-score sums), two
    tiny AllReduces, and a closed-form combination with f64 host constants.

The final scalar equals
  loss = (lsum_t - C2 - pos_t/T) / B + logZ1 + logZ2
with C2 = 2*B*K*log(c) - 2*TOT*log(c+eps), c = K/N_DATA, computed on host in
f64 (the naive formulation cancels 2.5e6-magnitude terms, which f32 cannot).
"""

import sys

sys.path.insert(0, "/opt/trn_rl_repo")

import math
import os

import numpy as np
import ml_dtypes

import concourse.bacc as bacc
import concourse.bass as bass
import concourse.mybir as mybir
import concourse.tile as tile
from concourse.bass_utils import run_bass_kernel_spmd

# ---- problem constants (from reference.py) ----
B = 128
K = 4096
KP1 = K + 1
D = 128
N_DATA = 500000
NCE_T = 0.07
EPS = 1e-7
FEAT = 2048

NCORES = 8
RPC = N_DATA // NCORES          # rows per core = 62500
HALF = RPC // 2                 # 31250, fits int16 indexing
TOT = B * KP1                   # 524416 sampled pairs per bank
C_NEG = K / N_DATA              # m * Pn
CP = C_NEG + EPS
GSEG = 8192                     # idxs per dma_gather call

F32 = mybir.dt.float32
BF16 = mybir.dt.bfloat16
I16 = mybir.dt.int16

_BUILD_CACHE = {}


def _wrap_idxs(idx_1d: np.ndarray) -> np.ndarray:
    """dma_gather int16 index layout: idx i -> [i % 16, i // 16], replicated
    into all 8 groups of 16 partitions."""
    n = idx_1d.shape[0]
    assert n % 16 == 0
    arr = np.zeros((128, n // 16), np.int16)
    base = idx_1d.reshape(-1, 16).T.astype(np.int16)
    for k in range(8):
        arr[16 * k:16 * k + 16, :] = base
    return arr


def _prep_host(inputs):
    f_s = np.asarray(inputs["f_s"], np.float32)
    f_t = np.asarray(inputs["f_t"], np.float32)
    idx = np.asarray(inputs["idx"]).astype(np.int64)
    cidx = np.asarray(inputs["contrast_idx"]).astype(np.int64)
    W_s = np.asarray(inputs["W_s"], np.float32)
    b_s = np.asarray(inputs["b_s"], np.float32)
    W_t = np.asarray(inputs["W_t"], np.float32)
    b_t = np.asarray(inputs["b_t"], np.float32)
    m1 = np.asarray(inputs["memory_v1"], np.float32)
    m2 = np.asarray(inputs["memory_v2"], np.float32)

    full_idx = np.concatenate([idx[:, None], cidx], axis=1)  # [B, KP1]
    b_of = np.broadcast_to(np.arange(B)[:, None], (B, KP1)).ravel()
    k_of = np.broadcast_to(np.arange(KP1)[None, :], (B, KP1)).ravel()
    r_of = full_idx.ravel()
    owner = r_of // RPC
    local = r_of - owner * RPC
    half = local // HALF
    sub = (local % HALF).astype(np.int64)

    # per (core, half): pairs sorted by b, b-runs padded to multiples of 128
    per = {}
    for m in range(NCORES):
        for hh in (0, 1):
            sel = (owner == m) & (half == hh)
            bs = b_of[sel]
            subs = sub[sel]
            poss = k_of[sel] == 0
            order = np.argsort(bs, kind="stable")
            bs, subs, poss = bs[order], subs[order], poss[order]
            starts = np.searchsorted(bs, np.arange(B + 1))
            rows_l, val_l, pos_l, cb_l = [], [], [], []
            for b in range(B):
                lo, hi = starts[b], starts[b + 1]
                n = hi - lo
                if n == 0:
                    continue
                pad = (-n) % 128
                rows_l.append(subs[lo:hi])
                if pad:
                    rows_l.append(np.zeros(pad, np.int64))
                val_l.append(np.concatenate([np.ones(n, np.float32),
                                             np.zeros(pad, np.float32)]))
                pos_l.append(np.concatenate([poss[lo:hi].astype(np.float32),
                                             np.zeros(pad, np.float32)]))
                cb_l.extend([b] * ((n + pad) // 128))
            rows = (np.concatenate(rows_l) if rows_l
                    else np.zeros(0, np.int64))
            val = (np.concatenate(val_l) if val_l
                   else np.zeros(0, np.float32))
            pos = (np.concatenate(pos_l) if pos_l
                   else np.zeros(0, np.float32))
            per[(m, hh)] = (rows, val, pos, np.array(cb_l, np.int64))

    nseg = max(len(per[k][0]) for k in per)
    nseg = -(-nseg // GSEG) * 128 if nseg % 128 else nseg  # round to 128
    nseg = -(-nseg // 128) * 128
    nch = nseg // 128
    ncht = 2 * nch
    ncsel = -(-ncht // 128) * 128

    emb_aug = {
        "fsT": np.concatenate([f_s.T, np.ones((1, B), np.float32)], 0),
        "ftT": np.concatenate([f_t.T, np.ones((1, B), np.float32)], 0),
        "Ws": np.concatenate([W_s, b_s[None, :]], 0),
        "Wt": np.concatenate([W_t, b_t[None, :]], 0),
    }

    m1b = m1.astype(ml_dtypes.bfloat16)
    m2b = m2.astype(ml_dtypes.bfloat16)

    in_maps = []
    for m in range(NCORES):
        imap = {k: v for k, v in emb_aug.items()}
        maskv = np.zeros((128, ncht), np.float32)
        maskp = np.zeros((128, ncht), np.float32)
        bsel = np.zeros(ncsel, np.int64)
        for hh in (0, 1):
            rows, val, pos, cb = per[(m, hh)]
            n = len(rows)
            rows_p = np.zeros(nseg, np.int64)
            rows_p[:n] = rows
            val_p = np.zeros(nseg, np.float32)
            val_p[:n] = val
            pos_p = np.zeros(nseg, np.float32)
            pos_p[:n] = pos
            imap[f"idx{hh}"] = _wrap_idxs(rows_p)
            maskv[:, hh * nch:(hh + 1) * nch] = val_p.reshape(nch, 128).T
            maskp[:, hh * nch:(hh + 1) * nch] = pos_p.reshape(nch, 128).T
            bsel[hh * nch: hh * nch + len(cb)] = cb
            base = m * RPC + hh * HALF
            imap[f"b1h{hh}"] = m1b[base:base + HALF]
            imap[f"b2h{hh}"] = m2b[base:base + HALF]
        imap["idxall"] = np.concatenate(
            [imap.pop("idx0"), imap.pop("idx1"), _wrap_idxs(bsel)], axis=1)
        imap["masks"] = np.concatenate([maskv, maskp], axis=1)
        in_maps.append(imap)

    c2 = 2.0 * B * K * math.log(C_NEG) - 2.0 * TOT * math.log(CP)
    return in_maps, nseg, ncht, ncsel, c2


def _build(nseg, ncht, ncsel, c2):
    key = (nseg, ncht, ncsel, round(c2, 6))
    if key in _BUILD_CACHE:
        return _BUILD_CACHE[key]
    nch = nseg // 128

    nc = bacc.Bacc(None, target_bir_lowering=False, debug=False)
    t_in = {}
    for nm in ("fsT", "ftT", "Ws", "Wt"):
        t_in[nm] = nc.dram_tensor(nm, [FEAT + 1, 128], F32, kind="ExternalInput")
    banks = {}
    for bk in (0, 1):
        for hh in (0, 1):
            nm = f"b{bk + 1}h{hh}"
            banks[(bk, hh)] = nc.dram_tensor(nm, [HALF, D], BF16,
                                             kind="ExternalInput")
    idxall_in = nc.dram_tensor("idxall", [128, (2 * nseg + ncsel) // 16], I16,
                               kind="ExternalInput")
    masks_in = nc.dram_tensor("masks", [128, 2 * ncht], F32,
                              kind="ExternalInput")
    out_ext = nc.dram_tensor("out", [1, 1], F32, kind="ExternalOutput")

    cc1_out = nc.dram_tensor("cc1_out_sh", [NCORES, 6], F32, addr_space="Shared")

    with tile.TileContext(nc) as tc:
        with (
            tc.tile_pool(name="persist", bufs=1) as pers,
            tc.tile_pool(name="work", bufs=3) as work,
            tc.tile_pool(name="gather", bufs=3) as gpool,
            tc.tile_pool(name="ps", bufs=2, space="PSUM") as ps,
            tc.tile_pool(name="ps1", bufs=1, space="PSUM") as ps1,
            tc.tile_pool(name="dram", bufs=1, space="DRAM") as dram,
        ):
            # ---- load small inputs (single DMA each) ----
            idxall = pers.tile([128, (2 * nseg + ncsel) // 16], I16,
                               tag="idxall")
            nc.sync.dma_start(idxall[:], idxall_in[:])
            idx_sb = {hh: idxall[:, hh * (nseg // 16):(hh + 1) * (nseg // 16)]
                      for hh in (0, 1)}
            bsel_sb = idxall[:, 2 * (nseg // 16):]
            masks = pers.tile([128, 2 * ncht], F32, tag="masks")
            nc.sync.dma_start(masks[:], masks_in[:])
            maskv = masks[:, 0:ncht]
            maskp = masks[:, ncht:2 * ncht]

            # ---- embeddings: e = l2norm(f @ W + b), then to DRAM (bf16) ----
            e_dram = {}
            nkf = FEAT // 128
            for which, fT_t, W_t_ in (("s", t_in["fsT"], t_in["Ws"]),
                                      ("t", t_in["ftT"], t_in["Wt"])):
                pe = ps.tile([128, 128], F32, space="PSUM", tag="emb_ps")
                fT_sb = work.tile([128, nkf, 128], F32, tag="emb_f")
                nc.sync.dma_start(
                    fT_sb[:], fT_t[0:FEAT, :].rearrange("(a p) c -> p a c", p=128))
                W_sb = work.tile([128, nkf, 128], F32, tag="emb_w")
                nc.sync.dma_start(
                    W_sb[:], W_t_[0:FEAT, :].rearrange("(a p) c -> p a c", p=128))
                fT_last = work.tile([1, 128], F32, tag="emb_fl")
                nc.sync.dma_start(fT_last[:], fT_t[FEAT:FEAT + 1, :])
                W_last = work.tile([1, 128], F32, tag="emb_wl")
                nc.sync.dma_start(W_last[:], W_t_[FEAT:FEAT + 1, :])
                for kk in range(nkf):
                    nc.tensor.matmul(out=pe[:], lhsT=fT_sb[:, kk, :],
                                     rhs=W_sb[:, kk, :],
                                     start=(kk == 0), stop=False)
                nc.tensor.matmul(out=pe[:], lhsT=fT_last[:], rhs=W_last[:],
                                 start=False, stop=True)
                sq = work.tile([128, 128], F32, tag="emb_sq")
                nc.scalar.square(sq[:], pe[:])
                ss = work.tile([128, 1], F32, tag="emb_ss")
                nc.vector.tensor_reduce(out=ss[:], in_=sq[:],
                                        axis=mybir.AxisListType.X,
                                        op=mybir.AluOpType.add)
                sr = work.tile([128, 1], F32, tag="emb_sr")
                nc.scalar.sqrt(sr[:], ss[:])
                ri = work.tile([128, 1], F32, tag="emb_ri")
                nc.vector.reciprocal(ri[:], sr[:])
                en = work.tile([128, 128], BF16, tag="emb_en")
                nc.vector.tensor_scalar_mul(en[:], pe[:], ri[:])
                ed = dram.tile([128, 128], BF16, tag=f"edram{which}")
                nc.sync.dma_start(ed[:], en[:])
                e_dram[which] = ed

            # ---- per-chunk embedding columns via tiny transposed gather ----
            # bank 0 = memory_v1 pairs with e_t; bank 1 = memory_v2 with e_s
            esel = {}
            for bk, which in ((0, "t"), (1, "s")):
                t = pers.tile([128, 1, ncsel], BF16, tag=f"esel{bk}")
                nc.gpsimd.dma_gather(
                    t[:], e_dram[which][:], bsel_sb[:], ncsel, ncsel, D,
                    elem_step=128, transpose=True, single_packet=False,
                )
                esel[bk] = t

            # ---- main: gather rows transposed, per-chunk dot via matmul ----
            packed = {bk: pers.tile([128, ncht], F32, tag=f"packed{bk}",
                                    name=f"packed{bk}")
                      for bk in (0, 1)}
            cur = {0: None, 1: None}

            def drain(bk):
                if cur[bk] is not None:
                    g, t = cur[bk]
                    lo = g * 512
                    n = min(512, ncht - lo)
                    nc.vector.tensor_copy(packed[bk][:, lo:lo + n], t[:, :n])
                    cur[bk] = None

            for bk in (0, 1):
                for hh in (0, 1):
                    for p0 in range(0, nseg, GSEG):
                        sz = min(GSEG, nseg - p0)
                        gt = gpool.tile([128, 1, GSEG], BF16, tag=f"gt{bk}")
                        nc.gpsimd.dma_gather(
                            gt[:, :, :sz], banks[(bk, hh)][:],
                            idx_sb[hh][:, p0 // 16:(p0 + sz) // 16],
                            sz, sz, D, elem_step=128, transpose=True,
                            single_packet=False,
                        )
                        for j2 in range(sz // 128):
                            c = hh * nch + p0 // 128 + j2
                            g = c // 512
                            if cur[bk] is None or cur[bk][0] != g:
                                drain(bk)
                                cur[bk] = (g, ps.tile([128, 512], F32,
                                                      space="PSUM",
                                                      tag=f"pk{bk}",
                                                      name=f"pk{bk}_{g}"))
                            nc.tensor.matmul(
                                out=cur[bk][1][:, c % 512:c % 512 + 1],
                                lhsT=gt[:, 0, j2 * 128:(j2 + 1) * 128],
                                rhs=esel[bk][:, 0, c:c + 1],
                                start=True, stop=True,
                            )
                drain(bk)

            # ---- local pass A + pass B (pass B uses the core-local Z
            # estimate Z0 = 8 * zsum_local * N/TOT; the induced error on
            # the summed log terms is ~1e-5 of the final loss), then ONE
            # AllGather of 6 partial sums ----
            ones_col = pers.tile([128, 1], F32, tag="ones_col")
            nc.vector.memset(ones_col[:], 1.0)
            ones_row = pers.tile([1, 128], F32, tag="ones_row")
            nc.vector.memset(ones_row[:], 1.0)

            # red6 cols: 0-1 zsum, 2-3 possum, 4-5 lsum
            red6 = pers.tile([128, 6], F32, tag="red6")
            u = {}
            zc_sb = pers.tile([1, 2], F32, tag="zc_sb")
            for bk in (0, 1):
                ut = pers.tile([128, ncht], F32, tag=f"u{bk}")
                nc.scalar.activation(ut[:], packed[bk][:],
                                     mybir.ActivationFunctionType.Exp,
                                     bias=0.0, scale=1.0 / NCE_T)
                u[bk] = ut
                scr = work.tile([128, ncht], F32, tag="scr")
                nc.vector.tensor_tensor(out=scr[:], in0=ut[:],
                                        in1=maskv,
                                        op=mybir.AluOpType.mult)
                nc.vector.tensor_reduce(
                    out=red6[:, bk:bk + 1], in_=scr[:],
                    axis=mybir.AxisListType.X, op=mybir.AluOpType.add)
                scr2 = work.tile([128, ncht], F32, tag="scr")
                nc.vector.tensor_tensor(out=scr2[:], in0=packed[bk][:],
                                        in1=maskp,
                                        op=mybir.AluOpType.mult)
                nc.vector.tensor_reduce(
                    out=red6[:, 2 + bk:3 + bk], in_=scr2[:],
                    axis=mybir.AxisListType.X, op=mybir.AluOpType.add)

                # local Z0: partition-reduce zsum, then 1/(Z0*cp) bcast
                psz = ps1.tile([1, 1], F32, space="PSUM", tag="red_ps")
                nc.tensor.matmul(out=psz[:], lhsT=ones_col[:],
                                 rhs=red6[:, bk:bk + 1],
                                 start=True, stop=True)
                nc.scalar.mul(zc_sb[:, bk:bk + 1], psz[:],
                              float(NCORES) * N_DATA / TOT)
                zi = work.tile([1, 1], F32, tag="zi")
                nc.vector.reciprocal(zi[:], zc_sb[:, bk:bk + 1])
                zs = work.tile([1, 1], F32, tag="zs")
                nc.vector.tensor_scalar_mul(zs[:], zi[:], 1.0 / CP)
                bc_ps = ps1.tile([128, 1], F32, space="PSUM", tag="bc_ps")
                nc.tensor.matmul(out=bc_ps[:], lhsT=ones_row[:],
                                 rhs=zs[:], start=True, stop=True)
                bc_sb = work.tile([128, 1], F32, tag="bcsb")
                nc.vector.tensor_copy(bc_sb[:], bc_ps[:])
                # ln1 = log(1 + u/(Z0*cp)) = log(u*zs + 1)
                ln1 = work.tile([128, ncht], F32, tag="ln1")
                nc.scalar.activation(ln1[:], ut[:],
                                     mybir.ActivationFunctionType.Ln,
                                     bias=1.0, scale=bc_sb[:])
                scr3 = work.tile([128, ncht], F32, tag="scr")
                nc.vector.tensor_tensor(out=scr3[:], in0=ln1[:],
                                        in1=maskv,
                                        op=mybir.AluOpType.mult)
                nc.vector.tensor_reduce(
                    out=red6[:, 4 + bk:5 + bk], in_=scr3[:],
                    axis=mybir.AxisListType.X, op=mybir.AluOpType.add)

            psr = ps1.tile([1, 6], F32, space="PSUM", tag="red_ps")
            nc.tensor.matmul(out=psr[:], lhsT=ones_col[:], rhs=red6[:],
                             start=True, stop=True)
            cc1_sb = work.tile([1, 6], F32, tag="cc1sb")
            nc.vector.tensor_copy(cc1_sb[:], psr[:])
            cc1_in = dram.tile([1, 6], F32, tag="cc1_in")
            nc.sync.dma_start(cc1_in[:], cc1_sb[:])
            nc.gpsimd.collective_compute(
                "AllGather", mybir.AluOpType.bypass,
                replica_groups=[list(range(NCORES))],
                ins=[cc1_in[:]], outs=[cc1_out[:]],
            )
            cc1g8 = work.tile([NCORES, 6], F32, tag="cc1g8")
            nc.sync.dma_start(cc1g8[:], cc1_out[:])
            psg1 = ps1.tile([1, 6], F32, space="PSUM", tag="red_ps")
            nc.tensor.matmul(out=psg1[:], lhsT=ones_col[0:NCORES, :],
                             rhs=cc1g8[:], start=True, stop=True)
            gsum = pers.tile([1, 6], F32, tag="gsum")
            nc.vector.tensor_copy(gsum[:], psg1[:])
            # gsum = (zsum1, zsum2, pos1, pos2, lsum1, lsum2) globals

            # logZ from exact global zsums
            zsc = pers.tile([1, 2], F32, tag="zsc")
            nc.scalar.mul(zsc[:], gsum[:, 0:2], float(N_DATA) / TOT)
            lzg = pers.tile([1, 2], F32, tag="lzg")
            nc.scalar.activation(lzg[:], zsc[:],
                                 mybir.ActivationFunctionType.Ln)

            # loss = (lsum_t - C2 - pos_t/T)/B + logZ1 + logZ2
            a_sb = work.tile([1, 1], F32, tag="fa")
            nc.vector.tensor_tensor(out=a_sb[:], in0=gsum[:, 4:5],
                                    in1=gsum[:, 5:6],
                                    op=mybir.AluOpType.add)
            p_sb = work.tile([1, 1], F32, tag="fp")
            nc.vector.tensor_tensor(out=p_sb[:], in0=gsum[:, 2:3],
                                    in1=gsum[:, 3:4],
                                    op=mybir.AluOpType.add)
            z_sb = work.tile([1, 1], F32, tag="fz")
            nc.vector.tensor_tensor(out=z_sb[:], in0=lzg[:, 0:1],
                                    in1=lzg[:, 1:2],
                                    op=mybir.AluOpType.add)
            q_sb = work.tile([1, 1], F32, tag="fq")
            nc.vector.tensor_scalar_mul(q_sb[:], p_sb[:], 1.0 / NCE_T)
            u_sb = work.tile([1, 1], F32, tag="fu")
            nc.vector.tensor_tensor(out=u_sb[:], in0=a_sb[:], in1=q_sb[:],
                                    op=mybir.AluOpType.subtract)
            v_sb = work.tile([1, 1], F32, tag="fv")
            nc.vector.tensor_scalar(out=v_sb[:], in0=u_sb[:],
                                    scalar1=float(c2), scalar2=None,
                                    op0=mybir.AluOpType.subtract)
            w_sb = work.tile([1, 1], F32, tag="fw")
            nc.vector.tensor_scalar_mul(w_sb[:], v_sb[:], 1.0 / B)
            loss_sb = work.tile([1, 1], F32, tag="floss")
            nc.vector.tensor_tensor(out=loss_sb[:], in0=w_sb[:],
                                    in1=z_sb[:],
                                    op=mybir.AluOpType.add)
            nc.sync.dma_start(out_ext[:], loss_sb[:])

    nc.compile()
    _BUILD_CACHE[key] = nc
    return nc


def run(inputs, trace=False):
    in_maps, nseg, ncht, ncsel, c2 = _prep_host(inputs)
    nc = _build(nseg, ncht, ncsel, c2)
    r = run_bass_kernel_spmd(nc, in_maps, list(range(NCORES)), trace=trace)
    loss = np.float32(r.results[0]["out"][0, 0])
    return loss.reshape(()), r


def kernel(**inputs) -> np.ndarray:
    loss, _ = run(inputs)
    return loss



# revision 7
# speedup vs baseline: 1.9530x; 1.9530x over previous
"""CRD (contrastive representation distillation) loss on 8 Trainium2 cores.

Strategy (memory-bound scatter-gather problem):
  - The two 500000x128 memory banks are interleaved row-wise on the host into
    one [500000, 256] bf16 array (m1_row || m2_row), sharded row-wise across
    the 8 cores (62500 rows each, split in two 31250-row halves so gather
    indices fit in int16). One 512-byte dma_gather descriptor then fetches
    BOTH banks' rows for a sampled pair at full DMA-bus efficiency.
  - The host lists, per (core, half), the sampled (b, k) pairs whose memory
    row falls in that shard, in natural b-sorted order with NO per-b padding.
    Each 128-pair chunk spans at most 2 distinct b values (b-runs are ~256
    long; a host-side packer inserts padding in the astronomically rare case
    a chunk would span 3). Per chunk the host ships (b_lo, b_hi) plus a
    per-pair "hi" mask selecting which of the two embedding columns applies.
  - On device, dma_gather(transpose=True) pulls the pair rows from HBM
    already transposed into SBUF tiles [d=128 partitions, 2, pairs]. Each
    128-pair chunk is the stationary matmul operand (lhsT) against the two
    candidate embedding columns; a masked select combines the two PSUM
    columns into the per-pair score.
  - Embeddings es/et (linear head + L2 norm) are computed on every core from
    host-prepacked bf16 f/W tiles. The per-chunk embedding columns are
    produced by a one-hot matmul (PE acts as the gather), keeping the Pool
    engine queue free for the main gather stream.
  - The loss reduces to six masked sums over the packed pair scores, one
    AllGather of 6 partial sums, and a closed-form combination with f64 host
    constants.

The final scalar equals
  loss = (lsum_t - C2 - pos_t/T) / B + logZ1 + logZ2
with C2 = 2*B*K*log(c) - 2*TOT*log(c+eps), c = K/N_DATA, computed on host in
f64 (the naive formulation cancels 2.5e6-magnitude terms, which f32 cannot).
"""

import sys

sys.path.insert(0, "/opt/trn_rl_repo")

import math

import numpy as np
import ml_dtypes

import concourse.bacc as bacc
import concourse.bass as bass
import concourse.mybir as mybir
import concourse.tile as tile
from concourse.bass_utils import run_bass_kernel_spmd

# ---- problem constants (from reference.py) ----
B = 128
K = 4096
KP1 = K + 1
D = 128
N_DATA = 500000
NCE_T = 0.07
EPS = 1e-7
FEAT = 2048

NCORES = 8
RPC = N_DATA // NCORES          # rows per core = 62500
HALF = RPC // 2                 # 31250, fits int16 indexing
TOT = B * KP1                   # 524416 sampled pairs per bank
C_NEG = K / N_DATA              # m * Pn
CP = C_NEG + EPS

F32 = mybir.dt.float32
BF16 = mybir.dt.bfloat16
I16 = mybir.dt.int16

_BUILD_CACHE = {}


def _wrap_idxs(idx_1d: np.ndarray) -> np.ndarray:
    """dma_gather int16 index layout: idx i -> [i % 16, i // 16], replicated
    into all 8 groups of 16 partitions."""
    n = idx_1d.shape[0]
    assert n % 16 == 0
    arr = np.zeros((128, n // 16), np.int16)
    base = idx_1d.reshape(-1, 16).T.astype(np.int16)
    for k in range(8):
        arr[16 * k:16 * k + 16, :] = base
    return arr


def _pack_stream(subs, bs, poss):
    """Pack one (core, half) pair stream: keep natural b-sorted order and pad
    (idx 0, val 0) only when a 128-chunk would otherwise span >2 distinct b.
    Returns per-pair (rows, val, pos, b)."""
    n = len(subs)
    # run-length encode b
    if n == 0:
        return (np.zeros(0, np.int64), np.zeros(0, np.float32),
                np.zeros(0, np.float32), np.zeros(0, np.int64))
    edges = np.flatnonzero(np.concatenate([[1], np.diff(bs) != 0]))
    starts = list(edges) + [n]
    out_r, out_v, out_p, out_b = [], [], [], []
    cur = 0
    last_b = 0
    for ri in range(len(starts) - 1):
        lo, hi = starts[ri], starts[ri + 1]
        bval = int(bs[lo])
        off = cur % 128
        if off:
            c0 = cur - off
            nb = len({int(x) for x in
                      np.concatenate(out_b)[c0:cur]}) if out_b else 0
            if nb >= 2:
                pad = 128 - off
                out_r.append(np.zeros(pad, np.int64))
                out_v.append(np.zeros(pad, np.float32))
                out_p.append(np.zeros(pad, np.float32))
                out_b.append(np.full(pad, last_b, np.int64))
                cur += pad
        cnt = hi - lo
        out_r.append(subs[lo:hi])
        out_v.append(np.ones(cnt, np.float32))
        out_p.append(poss[lo:hi].astype(np.float32))
        out_b.append(np.full(cnt, bval, np.int64))
        cur += cnt
        last_b = bval
    return (np.concatenate(out_r), np.concatenate(out_v),
            np.concatenate(out_p), np.concatenate(out_b))


def _prep_host(inputs):
    f_s = np.asarray(inputs["f_s"], np.float32)
    f_t = np.asarray(inputs["f_t"], np.float32)
    idx = np.asarray(inputs["idx"]).astype(np.int64)
    cidx = np.asarray(inputs["contrast_idx"]).astype(np.int64)
    W_s = np.asarray(inputs["W_s"], np.float32)
    b_s = np.asarray(inputs["b_s"], np.float32)
    W_t = np.asarray(inputs["W_t"], np.float32)
    b_t = np.asarray(inputs["b_t"], np.float32)
    m1 = np.asarray(inputs["memory_v1"], np.float32)
    m2 = np.asarray(inputs["memory_v2"], np.float32)

    full_idx = np.concatenate([idx[:, None], cidx], axis=1)  # [B, KP1]
    b_of = np.broadcast_to(np.arange(B)[:, None], (B, KP1)).ravel()
    k_of = np.broadcast_to(np.arange(KP1)[None, :], (B, KP1)).ravel()
    r_of = full_idx.ravel()
    owner = r_of // RPC
    local = r_of - owner * RPC
    half = local // HALF
    sub = (local % HALF).astype(np.int64)

    per = {}
    maxlen = 0
    for m in range(NCORES):
        for hh in (0, 1):
            sel = (owner == m) & (half == hh)
            packed = _pack_stream(sub[sel], b_of[sel], k_of[sel] == 0)
            per[(m, hh)] = packed
            maxlen = max(maxlen, len(packed[0]))

    # equal segments, each a multiple of 128, so gather tiles have one shape
    nsegments = -(-maxlen // 8192)
    seg = -(-maxlen // (nsegments * 128)) * 128
    nseg = nsegments * seg
    nch = nseg // 128
    ncht2 = 2 * nch
    ncsel = -(-2 * ncht2 // 128) * 128

    # prepacked embedding operands: [p, kk, c] = M[kk*128+p, c]
    def pack_pkc(M):
        return (M.reshape(16, 128, 128).transpose(1, 0, 2)
                .reshape(128, FEAT).astype(ml_dtypes.bfloat16))

    shared = {
        "fspack": pack_pkc(f_s.T.copy()),
        "ftpack": pack_pkc(f_t.T.copy()),
        "wspack": pack_pkc(W_s.copy()),
        "wtpack": pack_pkc(W_t.copy()),
        "bias2": np.stack([b_s, b_t]).astype(np.float32),
    }

    big = np.concatenate([m1.astype(ml_dtypes.bfloat16),
                          m2.astype(ml_dtypes.bfloat16)], axis=1)  # [N, 256]

    in_maps = []
    for m in range(NCORES):
        imap = dict(shared)
        masks = np.zeros((128, 3 * ncht2), np.float32)
        eselb = np.zeros(ncsel, np.int64)
        onehot = np.zeros((128, ncsel), ml_dtypes.bfloat16)
        idx_parts = []
        for hh in (0, 1):
            rows, val, pos, bofp = per[(m, hh)]
            n = len(rows)
            rows_p = np.zeros(nseg, np.int64)
            rows_p[:n] = rows
            val_p = np.zeros(nseg, np.float32)
            val_p[:n] = val
            pos_p = np.zeros(nseg, np.float32)
            pos_p[:n] = pos
            b_p = np.zeros(nseg, np.int64)
            b_p[:n] = bofp
            idx_parts.append(_wrap_idxs(rows_p))
            co = hh * nch
            masks[:, co:co + nch] = val_p.reshape(nch, 128).T
            masks[:, ncht2 + co:ncht2 + co + nch] = pos_p.reshape(nch, 128).T
            bmat = b_p.reshape(nch, 128)           # [chunk, pair-in-chunk]
            b_lo = bmat[:, 0]
            b_hi = bmat[:, -1]
            hi_mask = (bmat != b_lo[:, None]).astype(np.float32)
            masks[:, 2 * ncht2 + co:2 * ncht2 + co + nch] = hi_mask.T
            cg = np.arange(nch) + co
            eselb[2 * cg] = b_lo
            eselb[2 * cg + 1] = b_hi
            base = m * RPC + hh * HALF
            imap[f"bkh{hh}"] = big[base:base + HALF]
        onehot[eselb, np.arange(ncsel)] = 1.0
        imap["idxall"] = np.concatenate(
            idx_parts + [_wrap_idxs(eselb)], axis=1)
        imap["masks"] = masks
        imap["onehot"] = onehot
        in_maps.append(imap)

    c2 = 2.0 * B * K * math.log(C_NEG) - 2.0 * TOT * math.log(CP)
    return in_maps, nseg, seg, ncht2, ncsel, c2


def _build(nseg, seg, ncht2, ncsel, c2):
    key = (nseg, seg, ncht2, ncsel, round(c2, 6))
    if key in _BUILD_CACHE:
        return _BUILD_CACHE[key]
    nch = nseg // 128

    nc = bacc.Bacc(None, target_bir_lowering=False, debug=False)
    t_in = {}
    for nm in ("fspack", "ftpack", "wspack", "wtpack"):
        t_in[nm] = nc.dram_tensor(nm, [128, FEAT], BF16, kind="ExternalInput")
    bias2_in = nc.dram_tensor("bias2", [2, 128], F32, kind="ExternalInput")
    banks = {hh: nc.dram_tensor(f"bkh{hh}", [HALF, 2 * D], BF16,
                                kind="ExternalInput")
             for hh in (0, 1)}
    idxall_in = nc.dram_tensor("idxall", [128, (2 * nseg + ncsel) // 16], I16,
                               kind="ExternalInput")
    masks_in = nc.dram_tensor("masks", [128, 3 * ncht2], F32,
                              kind="ExternalInput")
    onehot_in = nc.dram_tensor("onehot", [128, ncsel], BF16,
                               kind="ExternalInput")
    out_ext = nc.dram_tensor("out", [1, 1], F32, kind="ExternalOutput")

    cc1_out = nc.dram_tensor("cc1_out_sh", [NCORES, 6], F32, addr_space="Shared")

    with tile.TileContext(nc) as tc:
        with (
            tc.tile_pool(name="persist", bufs=1) as pers,
            tc.tile_pool(name="work", bufs=3) as work,
            tc.tile_pool(name="gather", bufs=3) as gpool,
            tc.tile_pool(name="ps", bufs=2, space="PSUM") as ps,
            tc.tile_pool(name="ps1", bufs=1, space="PSUM") as ps1,
            tc.tile_pool(name="dram", bufs=1, space="DRAM") as dram,
        ):
            # ---- load small inputs; idxall first (it gates the gathers) ----
            idxall = pers.tile([128, (2 * nseg + ncsel) // 16], I16,
                               tag="idxall")
            nc.sync.dma_start(idxall[:], idxall_in[:])
            idx_sb = {hh: idxall[:, hh * (nseg // 16):(hh + 1) * (nseg // 16)]
                      for hh in (0, 1)}
            bsel_sb = idxall[:, 2 * (nseg // 16):]

            fw_sb = {}
            for nm in ("fspack", "ftpack", "wspack", "wtpack"):
                t = pers.tile([128, FEAT], BF16, tag=nm)
                nc.sync.dma_start(t[:], t_in[nm][:])
                fw_sb[nm] = t
            bias_sb = {}
            for i, which in ((0, "s"), (1, "t")):
                bt = pers.tile([1, 128], F32, tag=f"bias{which}")
                nc.sync.dma_start(bt[:], bias2_in[i:i + 1, :])
                bias_sb[which] = bt
            onehot_sb = pers.tile([128, ncsel], BF16, tag="onehot")
            nc.sync.dma_start(onehot_sb[:], onehot_in[:])
            masks = pers.tile([128, 3 * ncht2], F32, tag="masks")
            nc.sync.dma_start(masks[:], masks_in[:])
            maskv = masks[:, 0:ncht2]
            maskp = masks[:, ncht2:2 * ncht2]
            maskh = masks[:, 2 * ncht2:3 * ncht2]

            ones_col = pers.tile([128, 1], F32, tag="ones_col")
            nc.vector.memset(ones_col[:], 1.0)
            ones_row = pers.tile([1, 128], F32, tag="ones_row")
            nc.vector.memset(ones_row[:], 1.0)

            # ---- embeddings: e = l2norm(f @ W + b), kept in SBUF (bf16) ----
            en = {}
            for which, f_nm, w_nm in (("t", "ftpack", "wtpack"),
                                      ("s", "fspack", "wspack")):
                pe = ps.tile([128, 512], F32, space="PSUM", tag="scratch")
                pev = pe[:, 0:128]
                for kk in range(16):
                    sl = slice(kk * 128, (kk + 1) * 128)
                    nc.tensor.matmul(out=pev, lhsT=fw_sb[f_nm][:, sl],
                                     rhs=fw_sb[w_nm][:, sl],
                                     start=(kk == 0), stop=False)
                nc.tensor.matmul(out=pev, lhsT=ones_row[:],
                                 rhs=bias_sb[which][:],
                                 start=False, stop=True)
                sq = work.tile([128, 128], F32, tag="emb_sq")
                nc.scalar.square(sq[:], pev)
                ss = work.tile([128, 1], F32, tag="emb_ss")
                nc.vector.tensor_reduce(out=ss[:], in_=sq[:],
                                        axis=mybir.AxisListType.X,
                                        op=mybir.AluOpType.add)
                sr = work.tile([128, 1], F32, tag="emb_sr")
                nc.scalar.sqrt(sr[:], ss[:])
                ri = work.tile([128, 1], F32, tag="emb_ri")
                nc.vector.reciprocal(ri[:], sr[:])
                ent = pers.tile([128, 128], BF16, tag=f"en{which}")
                nc.vector.tensor_scalar_mul(ent[:], pev, ri[:])
                en[which] = ent

            # ---- per-chunk embedding columns via one-hot matmul (PE does
            # the gather; Pool queue stays free for the main gathers).
            # bank 0 = memory_v1 rows scored against e_t; bank 1 = m2 vs e_s.
            esel = pers.tile([128, 2, ncsel], BF16, tag="esel")
            for bk, which in ((0, "t"), (1, "s")):
                for blk in range(0, ncsel, 512):
                    bw = min(512, ncsel - blk)
                    pe2 = ps.tile([128, 512], F32, space="PSUM", tag="scratch")
                    nc.tensor.matmul(out=pe2[:, :bw], lhsT=en[which][:],
                                     rhs=onehot_sb[:, blk:blk + bw],
                                     start=True, stop=True)
                    nc.vector.tensor_copy(esel[:, bk, blk:blk + bw],
                                          pe2[:, :bw])

            # ---- main: gather interleaved pair rows (512B descriptors),
            # two 1-col matmuls per chunk per bank (lo/hi candidate e) ----
            packed = {(bk, w): pers.tile([128, ncht2], F32,
                                         tag=f"packed{bk}{w}",
                                         name=f"packed{bk}{w}")
                      for bk in (0, 1) for w in ("lo", "hi")}
            cur = {0: None, 1: None}

            def drain(bk):
                if cur[bk] is not None:
                    g, t = cur[bk]
                    lo = g * 256
                    n = min(256, ncht2 - lo)
                    nc.vector.tensor_copy(packed[(bk, "lo")][:, lo:lo + n],
                                          t[:, :n])
                    nc.vector.tensor_copy(packed[(bk, "hi")][:, lo:lo + n],
                                          t[:, 256:256 + n])
                    cur[bk] = None

            for hh in (0, 1):
                for p0 in range(0, nseg, seg):
                    gt = gpool.tile([128, 2, seg], BF16, tag="gt")
                    nc.gpsimd.dma_gather(
                        gt[:], banks[hh][:],
                        idx_sb[hh][:, p0 // 16:(p0 + seg) // 16],
                        seg, seg, 2 * D, elem_step=2 * D, transpose=True,
                        single_packet=False,
                    )
                    for j2 in range(seg // 128):
                        c = hh * nch + p0 // 128 + j2
                        g = c // 256
                        cc = c % 256
                        for bk in (0, 1):
                            if cur[bk] is None or cur[bk][0] != g:
                                drain(bk)
                                cur[bk] = (g, ps.tile([128, 512], F32,
                                                      space="PSUM",
                                                      tag=f"pk{bk}",
                                                      name=f"pk{bk}_{g}"))
                            lhsT = gt[:, bk, j2 * 128:(j2 + 1) * 128]
                            nc.tensor.matmul(
                                out=cur[bk][1][:, cc:cc + 1], lhsT=lhsT,
                                rhs=esel[:, bk, 2 * c:2 * c + 1],
                                start=True, stop=True)
                            nc.tensor.matmul(
                                out=cur[bk][1][:, 256 + cc:257 + cc],
                                lhsT=lhsT,
                                rhs=esel[:, bk, 2 * c + 1:2 * c + 2],
                                start=True, stop=True)
            drain(0)
            drain(1)

            # ---- select lo/hi, then local pass A + pass B (pass B uses the
            # core-local Z estimate Z0 = 8 * zsum_local * N/TOT; the induced
            # error on the summed log terms is ~1e-5 of the final loss),
            # then ONE AllGather of 6 partial sums ----
            red6 = pers.tile([128, 6], F32, tag="red6")
            zc_sb = pers.tile([1, 2], F32, tag="zc_sb")
            for bk in (0, 1):
                dt_ = work.tile([128, ncht2], F32, tag="seld")
                nc.vector.tensor_tensor(out=dt_[:], in0=packed[(bk, "hi")][:],
                                        in1=packed[(bk, "lo")][:],
                                        op=mybir.AluOpType.subtract)
                dm = work.tile([128, ncht2], F32, tag="seldm")
                nc.vector.tensor_tensor(out=dm[:], in0=dt_[:], in1=maskh,
                                        op=mybir.AluOpType.mult)
                sel = work.tile([128, ncht2], F32, tag="selv")
                nc.vector.tensor_tensor(out=sel[:], in0=packed[(bk, "lo")][:],
                                        in1=dm[:], op=mybir.AluOpType.add)

                ut = pers.tile([128, ncht2], F32, tag=f"u{bk}")
                nc.scalar.activation(ut[:], sel[:],
                                     mybir.ActivationFunctionType.Exp,
                                     bias=0.0, scale=1.0 / NCE_T)
                scr = work.tile([128, ncht2], F32, tag="scr")
                nc.vector.tensor_tensor(out=scr[:], in0=ut[:], in1=maskv,
                                        op=mybir.AluOpType.mult)
                nc.vector.tensor_reduce(
                    out=red6[:, bk:bk + 1], in_=scr[:],
                    axis=mybir.AxisListType.X, op=mybir.AluOpType.add)
                scr2 = work.tile([128, ncht2], F32, tag="scr")
                nc.vector.tensor_tensor(out=scr2[:], in0=sel[:], in1=maskp,
                                        op=mybir.AluOpType.mult)
                nc.vector.tensor_reduce(
                    out=red6[:, 2 + bk:3 + bk], in_=scr2[:],
                    axis=mybir.AxisListType.X, op=mybir.AluOpType.add)

                # local Z0: partition-reduce zsum, then 1/(Z0*cp) bcast
                psz = ps1.tile([1, 1], F32, space="PSUM", tag="red_ps")
                nc.tensor.matmul(out=psz[:], lhsT=ones_col[:],
                                 rhs=red6[:, bk:bk + 1],
                                 start=True, stop=True)
                nc.scalar.mul(zc_sb[:, bk:bk + 1], psz[:],
                              float(NCORES) * N_DATA / TOT)
                zi = work.tile([1, 1], F32, tag="zi")
                nc.vector.reciprocal(zi[:], zc_sb[:, bk:bk + 1])
                zs = work.tile([1, 1], F32, tag="zs")
                nc.vector.tensor_scalar_mul(zs[:], zi[:], 1.0 / CP)
                bc_ps = ps1.tile([128, 1], F32, space="PSUM", tag="bc_ps")
                nc.tensor.matmul(out=bc_ps[:], lhsT=ones_row[:],
                                 rhs=zs[:], start=True, stop=True)
                bc_sb = work.tile([128, 1], F32, tag="bcsb")
                nc.vector.tensor_copy(bc_sb[:], bc_ps[:])
                # ln1 = log(1 + u/(Z0*cp)) = log(u*zs + 1)
                ln1 = work.tile([128, ncht2], F32, tag="ln1")
                nc.scalar.activation(ln1[:], ut[:],
                                     mybir.ActivationFunctionType.Ln,
                                     bias=1.0, scale=bc_sb[:])
                scr3 = work.tile([128, ncht2], F32, tag="scr")
                nc.vector.tensor_tensor(out=scr3[:], in0=ln1[:], in1=maskv,
                                        op=mybir.AluOpType.mult)
                nc.vector.tensor_reduce(
                    out=red6[:, 4 + bk:5 + bk], in_=scr3[:],
                    axis=mybir.AxisListType.X, op=mybir.AluOpType.add)

            psr = ps1.tile([1, 6], F32, space="PSUM", tag="red_ps")
            nc.tensor.matmul(out=psr[:], lhsT=ones_col[:], rhs=red6[:],
                             start=True, stop=True)
            cc1_sb = work.tile([1, 6], F32, tag="cc1sb")
            nc.vector.tensor_copy(cc1_sb[:], psr[:])
            cc1_in = dram.tile([1, 6], F32, tag="cc1_in")
            nc.sync.dma_start(cc1_in[:], cc1_sb[:])
            nc.gpsimd.collective_compute(
                "AllGather", mybir.AluOpType.bypass,
                replica_groups=[list(range(NCORES))],
                ins=[cc1_in[:]], outs=[cc1_out[:]],
            )
            cc1g8 = work.tile([NCORES, 6], F32, tag="cc1g8")
            nc.sync.dma_start(cc1g8[:], cc1_out[:])
            psg1 = ps1.tile([1, 6], F32, space="PSUM", tag="red_ps")
            nc.tensor.matmul(out=psg1[:], lhsT=ones_col[0:NCORES, :],
                             rhs=cc1g8[:], start=True, stop=True)
            gsum = pers.tile([1, 6], F32, tag="gsum")
            nc.vector.tensor_copy(gsum[:], psg1[:])
            # gsum = (zsum1, zsum2, pos1, pos2, lsum1, lsum2) globals

            # logZ from exact global zsums
            zsc = pers.tile([1, 2], F32, tag="zsc")
            nc.scalar.mul(zsc[:], gsum[:, 0:2], float(N_DATA) / TOT)
            lzg = pers.tile([1, 2], F32, tag="lzg")
            nc.scalar.activation(lzg[:], zsc[:],
                                 mybir.ActivationFunctionType.Ln)

            # loss = (lsum_t - C2 - pos_t/T)/B + logZ1 + logZ2
            a_sb = work.tile([1, 1], F32, tag="fa")
            nc.vector.tensor_tensor(out=a_sb[:], in0=gsum[:, 4:5],
                                    in1=gsum[:, 5:6],
                                    op=mybir.AluOpType.add)
            p_sb = work.tile([1, 1], F32, tag="fp")
            nc.vector.tensor_tensor(out=p_sb[:], in0=gsum[:, 2:3],
                                    in1=gsum[:, 3:4],
                                    op=mybir.AluOpType.add)
            z_sb = work.tile([1, 1], F32, tag="fz")
            nc.vector.tensor_tensor(out=z_sb[:], in0=lzg[:, 0:1],
                                    in1=lzg[:, 1:2],
                                    op=mybir.AluOpType.add)
            q_sb = work.tile([1, 1], F32, tag="fq")
            nc.vector.tensor_scalar_mul(q_sb[:], p_sb[:], 1.0 / NCE_T)
            u_sb = work.tile([1, 1], F32, tag="fu")
            nc.vector.tensor_tensor(out=u_sb[:], in0=a_sb[:], in1=q_sb[:],
                                    op=mybir.AluOpType.subtract)
            v_sb = work.tile([1, 1], F32, tag="fv")
            nc.vector.tensor_scalar(out=v_sb[:], in0=u_sb[:],
                                    scalar1=float(c2), scalar2=None,
                                    op0=mybir.AluOpType.subtract)
            w_sb = work.tile([1, 1], F32, tag="fw")
            nc.vector.tensor_scalar_mul(w_sb[:], v_sb[:], 1.0 / B)
            loss_sb = work.tile([1, 1], F32, tag="floss")
            nc.vector.tensor_tensor(out=loss_sb[:], in0=w_sb[:],
                                    in1=z_sb[:],
                                    op=mybir.AluOpType.add)
            nc.sync.dma_start(out_ext[:], loss_sb[:])

    nc.compile()
    _BUILD_CACHE[key] = nc
    return nc


def run(inputs, trace=False):
    in_maps, nseg, seg, ncht2, ncsel, c2 = _prep_host(inputs)
    nc = _build(nseg, seg, ncht2, ncsel, c2)
    r = run_bass_kernel_spmd(nc, in_maps, list(range(NCORES)), trace=trace)
    loss = np.float32(r.results[0]["out"][0, 0])
    return loss.reshape(()), r


def kernel(**inputs) -> np.ndarray:
    loss, _ = run(inputs)
    return loss


# revision 10
# speedup vs baseline: 2.1691x; 1.1107x over previous
"""CRD (contrastive representation distillation) loss on 8 Trainium2 cores.

Strategy (memory-bound scatter-gather problem):
  - The two 500000x128 memory banks are interleaved row-wise on the host into
    one [500000, 256] bf16 array (m1_row || m2_row), sharded row-wise across
    the 8 cores (62500 rows each, split in two 31250-row halves so gather
    indices fit in int16). One 512-byte dma_gather descriptor then fetches
    BOTH banks' rows for a sampled pair at full DMA-bus efficiency.
  - The host lists, per (core, half), the sampled (b, k) pairs whose memory
    row falls in that shard, in natural b-sorted order with NO per-b padding.
    Each 128-pair chunk spans at most 2 distinct b values (b-runs are ~256
    long; the host packer inserts padding in the astronomically rare case a
    chunk would span 3). Per chunk the host ships the two candidate batch
    ids (as a one-hot matrix) plus a per-pair "hi" mask; padded slots carry
    an additive -2^20 score offset so exp() kills them and ln(1+0)=0.
  - On device, dma_gather(transpose=True) pulls the pair rows from HBM
    already transposed into SBUF tiles [d=128 partitions, 2, pairs]. Each
    128-pair chunk is the stationary matmul operand (lhsT) against the two
    candidate embedding columns; copy_predicated selects hi vs lo.
  - Embeddings es/et (linear head + L2 norm) are computed on every core from
    host-prepacked fp8 f/W tiles (per-tensor scales cancel in the L2 norm).
    Per-chunk embedding columns come from a one-hot matmul (the PE acts as
    the gather), keeping the Pool engine free for the main gather stream.
    1/sqrt(ss) is computed as exp(-0.5*ln(ss)) so every activation lives in
    one table set (no mid-kernel table reloads).
  - Score groups are reduced as soon as their PSUM group fills, overlapped
    with the gather stream (exp's accum_out gives the masked zsum for free).
    The tail is only: last tiny group, the local-Z chain, two ln passes with
    accum_out, one AllGather of 6 partial sums, and a coefficient dot.

The final scalar equals
  loss = (lsum_t - C2 - pos_t/T) / B + logZ1 + logZ2
with C2 = 2*B*K*log(c) - 2*TOT*log(c+eps), c = K/N_DATA, computed on host in
f64 (the naive formulation cancels 2.5e6-magnitude terms, which f32 cannot).
"""

import sys

sys.path.insert(0, "/opt/trn_rl_repo")

import math

import numpy as np
import ml_dtypes

import concourse.bacc as bacc
import concourse.bass as bass
import concourse.mybir as mybir
import concourse.tile as tile
from concourse.bass_utils import run_bass_kernel_spmd

# ---- problem constants (from reference.py) ----
B = 128
K = 4096
KP1 = K + 1
D = 128
N_DATA = 500000
NCE_T = 0.07
EPS = 1e-7
FEAT = 2048

NCORES = 8
RPC = N_DATA // NCORES          # rows per core = 62500
HALF = RPC // 2                 # 31250, fits int16 indexing
TOT = B * KP1                   # 524416 sampled pairs per bank
C_NEG = K / N_DATA              # m * Pn
CP = C_NEG + EPS
PAD_OFF = -1048576.0            # additive score offset for padded slots

F32 = mybir.dt.float32
BF16 = mybir.dt.bfloat16
FP8 = mybir.dt.float8e4
I16 = mybir.dt.int16
NPFP8 = ml_dtypes.float8_e4m3

F_SCALE = 8.0                   # cancels in l2norm; avoids fp8 subnormals
W_SCALE = 32.0

_BUILD_CACHE = {}


def _wrap_idxs(idx_1d: np.ndarray) -> np.ndarray:
    """dma_gather int16 index layout: idx i -> [i % 16, i // 16], replicated
    into all 8 groups of 16 partitions."""
    n = idx_1d.shape[0]
    assert n % 16 == 0
    arr = np.zeros((128, n // 16), np.int16)
    base = idx_1d.reshape(-1, 16).T.astype(np.int16)
    for k in range(8):
        arr[16 * k:16 * k + 16, :] = base
    return arr


def _pack_stream(subs, bs, poss):
    """Pack one (core, half) pair stream: keep natural b-sorted order and pad
    (idx 0, val 0) only when a 128-chunk would otherwise span >2 distinct b.
    Returns per-pair (rows, val, pos, b)."""
    n = len(subs)
    if n == 0:
        return (np.zeros(0, np.int64), np.zeros(0, np.float32),
                np.zeros(0, np.float32), np.zeros(0, np.int64))
    edges = np.flatnonzero(np.concatenate([[1], np.diff(bs) != 0]))
    starts = list(edges) + [n]
    out_r, out_v, out_p, out_b = [], [], [], []
    cur = 0
    last_b = 0
    for ri in range(len(starts) - 1):
        lo, hi = starts[ri], starts[ri + 1]
        bval = int(bs[lo])
        off = cur % 128
        if off:
            c0 = cur - off
            nb = len({int(x) for x in
                      np.concatenate(out_b)[c0:cur]}) if out_b else 0
            if nb >= 2:
                pad = 128 - off
                out_r.append(np.zeros(pad, np.int64))
                out_v.append(np.zeros(pad, np.float32))
                out_p.append(np.zeros(pad, np.float32))
                out_b.append(np.full(pad, last_b, np.int64))
                cur += pad
        cnt = hi - lo
        out_r.append(subs[lo:hi])
        out_v.append(np.ones(cnt, np.float32))
        out_p.append(poss[lo:hi].astype(np.float32))
        out_b.append(np.full(cnt, bval, np.int64))
        cur += cnt
        last_b = bval
    return (np.concatenate(out_r), np.concatenate(out_v),
            np.concatenate(out_p), np.concatenate(out_b))


def _prep_host(inputs):
    f_s = np.asarray(inputs["f_s"], np.float32)
    f_t = np.asarray(inputs["f_t"], np.float32)
    idx = np.asarray(inputs["idx"]).astype(np.int64)
    cidx = np.asarray(inputs["contrast_idx"]).astype(np.int64)
    W_s = np.asarray(inputs["W_s"], np.float32)
    b_s = np.asarray(inputs["b_s"], np.float32)
    W_t = np.asarray(inputs["W_t"], np.float32)
    b_t = np.asarray(inputs["b_t"], np.float32)
    m1 = np.asarray(inputs["memory_v1"], np.float32)
    m2 = np.asarray(inputs["memory_v2"], np.float32)

    full_idx = np.concatenate([idx[:, None], cidx], axis=1)  # [B, KP1]
    b_of = np.broadcast_to(np.arange(B)[:, None], (B, KP1)).ravel()
    k_of = np.broadcast_to(np.arange(KP1)[None, :], (B, KP1)).ravel()
    r_of = full_idx.ravel()
    owner = r_of // RPC
    local = r_of - owner * RPC
    half = local // HALF
    sub = (local % HALF).astype(np.int64)

    per = {}
    maxlen = 0
    for m in range(NCORES):
        for hh in (0, 1):
            sel = (owner == m) & (half == hh)
            packed = _pack_stream(sub[sel], b_of[sel], k_of[sel] == 0)
            per[(m, hh)] = packed
            maxlen = max(maxlen, len(packed[0]))

    # equal segments (mult of 128); the last segment of half 1 is split off
    # small so the final PSUM group finalizes in ~1us
    nsegments = -(-maxlen // 8192)
    seg = -(-maxlen // (nsegments * 128)) * 128
    nseg = nsegments * seg
    nch = nseg // 128
    ncht2 = 2 * nch
    ncsel = -(-2 * ncht2 // 128) * 128

    # prepacked embedding operands: [p, kk, c] = M[kk*128+p, c], fp8 w/ scale
    def pack_pkc(M, scale):
        return ((M * scale).reshape(16, 128, 128).transpose(1, 0, 2)
                .reshape(128, FEAT).astype(NPFP8))

    shared = {
        "fspack": pack_pkc(f_s.T.copy(), F_SCALE),
        "ftpack": pack_pkc(f_t.T.copy(), F_SCALE),
        "wspack": pack_pkc(W_s.copy(), W_SCALE),
        "wtpack": pack_pkc(W_t.copy(), W_SCALE),
        "bias2": (np.stack([b_s, b_t]) * F_SCALE * W_SCALE).astype(NPFP8),
    }

    big = np.concatenate([m1.astype(ml_dtypes.bfloat16),
                          m2.astype(ml_dtypes.bfloat16)], axis=1)  # [N, 256]

    in_maps = []
    for m in range(NCORES):
        imap = dict(shared)
        masks = np.zeros((128, 2 * ncht2), ml_dtypes.bfloat16)
        maskh8 = np.zeros((128, ncht2), np.uint8)
        eselb = np.zeros(ncsel, np.int64)
        onehot = np.zeros((128, ncsel), ml_dtypes.bfloat16)
        idx_parts = []
        for hh in (0, 1):
            rows, val, pos, bofp = per[(m, hh)]
            n = len(rows)
            rows_p = np.zeros(nseg, np.int64)
            rows_p[:n] = rows
            val_p = np.zeros(nseg, np.float32)
            val_p[:n] = val
            pos_p = np.zeros(nseg, np.float32)
            pos_p[:n] = pos
            b_p = np.zeros(nseg, np.int64)
            b_p[:n] = bofp
            idx_parts.append(_wrap_idxs(rows_p))
            co = hh * nch
            madd = np.where(val_p > 0, 0.0, PAD_OFF).astype(np.float32)
            masks[:, co:co + nch] = madd.reshape(nch, 128).T
            masks[:, ncht2 + co:ncht2 + co + nch] = pos_p.reshape(nch, 128).T
            bmat = b_p.reshape(nch, 128)           # [chunk, pair-in-chunk]
            b_lo = bmat[:, 0]
            b_hi = bmat[:, -1]
            hi_mask = ((bmat != b_lo[:, None]) &
                       (val_p.reshape(nch, 128) > 0)).astype(np.uint8)
            maskh8[:, co:co + nch] = hi_mask.T
            cg = np.arange(nch) + co
            eselb[2 * cg] = b_lo
            eselb[2 * cg + 1] = b_hi
            base = m * RPC + hh * HALF
            imap[f"bkh{hh}"] = big[base:base + HALF]
        onehot[eselb, np.arange(ncsel)] = 1.0
        imap["idxall"] = np.concatenate(idx_parts, axis=1)
        imap["masks"] = masks
        imap["maskh8"] = maskh8
        imap["onehot"] = onehot
        in_maps.append(imap)

    c2 = 2.0 * B * K * math.log(C_NEG) - 2.0 * TOT * math.log(CP)
    return in_maps, nseg, seg, ncht2, ncsel, c2


def _build(nseg, seg, ncht2, ncsel, c2):
    key = (nseg, seg, ncht2, ncsel, round(c2, 6))
    if key in _BUILD_CACHE:
        return _BUILD_CACHE[key]
    nch = nseg // 128

    # segment plan: [ (half, chunk0, nchunks) ... ]; split the very last one
    segs = []
    for hh in (0, 1):
        plan = [seg] * (nseg // seg)
        if hh == 1 and seg > 1024:
            plan = plan[:-1] + [seg - 1024, 1024]
        p0 = 0
        for sz in plan:
            segs.append((hh, p0, sz))
            p0 += sz
    last_nch = segs[-1][2] // 128

    # chunk groups (PSUM drains): <=256 chunks, last segment its own group
    cut = ncht2 - last_nch
    bounds = list(range(0, cut, 256)) + [cut, ncht2]
    bounds = sorted(set(bounds))
    NG = len(bounds) - 1
    grp_of = np.zeros(ncht2, np.int64)
    for gi in range(NG):
        grp_of[bounds[gi]:bounds[gi + 1]] = gi

    nc = bacc.Bacc(None, target_bir_lowering=False, debug=False)
    t_in = {}
    for nm in ("fspack", "ftpack", "wspack", "wtpack"):
        t_in[nm] = nc.dram_tensor(nm, [128, FEAT], FP8, kind="ExternalInput")
    bias2_in = nc.dram_tensor("bias2", [2, 128], FP8, kind="ExternalInput")
    banks = {hh: nc.dram_tensor(f"bkh{hh}", [HALF, 2 * D], BF16,
                                kind="ExternalInput")
             for hh in (0, 1)}
    idxall_in = nc.dram_tensor("idxall", [128, 2 * nseg // 16], I16,
                               kind="ExternalInput")
    masks_in = nc.dram_tensor("masks", [128, 2 * ncht2], BF16,
                              kind="ExternalInput")
    maskh_in = nc.dram_tensor("maskh8", [128, ncht2], mybir.dt.uint8,
                              kind="ExternalInput")
    onehot_in = nc.dram_tensor("onehot", [128, ncsel], BF16,
                               kind="ExternalInput")
    out_ext = nc.dram_tensor("out", [1, 1], F32, kind="ExternalOutput")

    cc1_out = nc.dram_tensor("cc1_out_sh", [NCORES, 6], F32, addr_space="Shared")

    with tile.TileContext(nc) as tc:
        with (
            tc.tile_pool(name="persist", bufs=1) as pers,
            tc.tile_pool(name="work", bufs=3) as work,
            tc.tile_pool(name="gather", bufs=3) as gpool,
            tc.tile_pool(name="gather1", bufs=1) as gpool1,
            tc.tile_pool(name="ps", bufs=2, space="PSUM") as ps,
            tc.tile_pool(name="ps1", bufs=1, space="PSUM") as ps1,
            tc.tile_pool(name="dram", bufs=1, space="DRAM") as dram,
        ):
            # ---- load small inputs; idxall first (it gates the gathers) ----
            idxall = pers.tile([128, 2 * nseg // 16], I16, tag="idxall")
            nc.sync.dma_start(idxall[:], idxall_in[:])
            idx_sb = {hh: idxall[:, hh * (nseg // 16):(hh + 1) * (nseg // 16)]
                      for hh in (0, 1)}

            fw_sb = {}
            for nm in ("fspack", "ftpack", "wspack", "wtpack"):
                t = pers.tile([128, FEAT], FP8, tag=nm)
                nc.sync.dma_start(t[:], t_in[nm][:])
                fw_sb[nm] = t
            bias_sb = {}
            for i, which in ((0, "s"), (1, "t")):
                bt = pers.tile([1, 128], FP8, tag=f"bias{which}")
                nc.sync.dma_start(bt[:], bias2_in[i:i + 1, :])
                bias_sb[which] = bt
            onehot_sb = pers.tile([128, ncsel], BF16, tag="onehot")
            nc.sync.dma_start(onehot_sb[:], onehot_in[:])
            masks_bf = pers.tile([128, 2 * ncht2], BF16, tag="masksbf")
            nc.sync.dma_start(masks_bf[:], masks_in[:])
            maskh_sb = pers.tile([128, ncht2], mybir.dt.uint8, tag="maskh")
            nc.sync.dma_start(maskh_sb[:], maskh_in[:])
            masksf = pers.tile([128, 2 * ncht2], F32, tag="masksf")
            nc.vector.tensor_copy(masksf[:], masks_bf[:])
            madd = masksf[:, 0:ncht2]
            maskp = masksf[:, ncht2:2 * ncht2]

            ones_col = pers.tile([128, 1], F32, tag="ones_col")
            nc.vector.memset(ones_col[:], 1.0)
            ones_row8 = pers.tile([1, 128], FP8, tag="ones_row8")
            nc.vector.memset(ones_row8[:], 1.0)
            ones_row = pers.tile([1, 128], F32, tag="ones_row")
            nc.vector.memset(ones_row[:], 1.0)
            redparts = pers.tile([128, 6, NG], F32, tag="redparts")
            nc.vector.memset(redparts[:], 0.0)
            g8 = pers.tile([1, 6], F32, tag="g8")
            nc.vector.memset(g8[:], 0.0)
            coef6 = pers.tile([1, 6], F32, tag="coef6")
            nc.vector.memset(coef6[:, 0:2], -1.0 / (B * NCE_T))
            nc.vector.memset(coef6[:, 2:4], 1.0 / B)
            nc.vector.memset(coef6[:, 4:6], 1.0)

            # ---- embeddings: e = l2norm(f @ W + b), kept in SBUF (bf16);
            # 1/sqrt via exp(-0.5*ln) so only one act table set is used ----
            en = {}
            for which, f_nm, w_nm in (("t", "ftpack", "wtpack"),
                                      ("s", "fspack", "wspack")):
                pe = ps.tile([128, 512], F32, space="PSUM", tag="scratch")
                pev = pe[:, 0:128]
                for kk in range(16):
                    sl = slice(kk * 128, (kk + 1) * 128)
                    nc.tensor.matmul(out=pev, lhsT=fw_sb[f_nm][:, sl],
                                     rhs=fw_sb[w_nm][:, sl],
                                     start=(kk == 0), stop=False)
                nc.tensor.matmul(out=pev, lhsT=ones_row8[:],
                                 rhs=bias_sb[which][:],
                                 start=False, stop=True)
                sq = work.tile([128, 128], F32, tag="emb_sq")
                nc.scalar.square(sq[:], pev)
                ss = work.tile([128, 1], F32, tag="emb_ss")
                nc.vector.tensor_reduce(out=ss[:], in_=sq[:],
                                        axis=mybir.AxisListType.X,
                                        op=mybir.AluOpType.add)
                lns = work.tile([128, 1], F32, tag="emb_lns")
                nc.scalar.activation(lns[:], ss[:],
                                     mybir.ActivationFunctionType.Ln)
                ri = work.tile([128, 1], F32, tag="emb_ri")
                nc.scalar.activation(ri[:], lns[:],
                                     mybir.ActivationFunctionType.Exp,
                                     bias=0.0, scale=-0.5)
                ent = pers.tile([128, 128], BF16, tag=f"en{which}")
                nc.vector.tensor_scalar_mul(ent[:], pev, ri[:])
                en[which] = ent

            # ---- per-chunk embedding columns via one-hot matmul ----
            # bank 0 = memory_v1 rows scored against e_t; bank 1 = m2 vs e_s
            esel = pers.tile([128, 2, ncsel], BF16, tag="esel")
            for bk, which in ((0, "t"), (1, "s")):
                for blk in range(0, ncsel, 512):
                    bw = min(512, ncsel - blk)
                    pe2 = ps.tile([128, 512], F32, space="PSUM", tag="scratch")
                    nc.tensor.matmul(out=pe2[:, :bw], lhsT=en[which][:],
                                     rhs=onehot_sb[:, blk:blk + bw],
                                     start=True, stop=True)
                    nc.vector.tensor_copy(esel[:, bk, blk:blk + bw],
                                          pe2[:, :bw])

            # ---- main loop: gather interleaved pair rows (512B
            # descriptors), two 1-col matmuls per chunk per bank, and
            # finalize each PSUM group as soon as it fills ----
            selbig = pers.tile([128, 2, ncht2], F32, tag="selbig")
            ubig = pers.tile([128, 2, ncht2], F32, tag="ubig")
            cur = {0: None, 1: None}

            def finalize(bk):
                if cur[bk] is None:
                    return
                g, t = cur[bk]
                lo = bounds[g]
                n = bounds[g + 1] - lo
                sv = selbig[:, bk, lo:lo + n]
                # sel = (pk_lo + pad_offset), overwritten with pk_hi where
                # the hi-mask is set (pads have maskh=0, keeping the offset)
                nc.vector.tensor_tensor(out=sv, in0=t[:, :n],
                                        in1=madd[:, lo:lo + n],
                                        op=mybir.AluOpType.add)
                nc.vector.copy_predicated(sv, maskh_sb[:, lo:lo + n],
                                          t[:, 256:256 + n])
                # u = exp(sel/T); accum_out = masked zsum partial for free
                nc.scalar.activation(ubig[:, bk, lo:lo + n], sv,
                                     mybir.ActivationFunctionType.Exp,
                                     bias=0.0, scale=1.0 / NCE_T,
                                     accum_out=redparts[:, bk, g:g + 1])
                scr = work.tile([128, 256], F32, tag="pscr")
                nc.vector.tensor_tensor(out=scr[:, :n], in0=sv,
                                        in1=maskp[:, lo:lo + n],
                                        op=mybir.AluOpType.mult)
                nc.vector.tensor_reduce(
                    out=redparts[:, 2 + bk, g:g + 1], in_=scr[:, :n],
                    axis=mybir.AxisListType.X, op=mybir.AluOpType.add)
                cur[bk] = None

            for hh, p0, sz in segs:
                pool = gpool if sz == seg else gpool1
                gt = pool.tile([128, 2, sz], BF16, tag=f"gt{sz}",
                               name=f"gt{hh}_{p0}")
                nc.gpsimd.dma_gather(
                    gt[:], banks[hh][:],
                    idx_sb[hh][:, p0 // 16:(p0 + sz) // 16],
                    sz, sz, 2 * D, elem_step=2 * D, transpose=True,
                    single_packet=False,
                )
                for j2 in range(sz // 128):
                    c = hh * nch + p0 // 128 + j2
                    g = int(grp_of[c])
                    cc = c - bounds[g]
                    for bk in (0, 1):
                        if cur[bk] is None or cur[bk][0] != g:
                            finalize(bk)
                            cur[bk] = (g, ps.tile([128, 512], F32,
                                                  space="PSUM",
                                                  tag=f"pk{bk}",
                                                  name=f"pk{bk}_{g}"))
                        lhsT = gt[:, bk, j2 * 128:(j2 + 1) * 128]
                        nc.tensor.matmul(
                            out=cur[bk][1][:, cc:cc + 1], lhsT=lhsT,
                            rhs=esel[:, bk, 2 * c:2 * c + 1],
                            start=True, stop=True)
                        nc.tensor.matmul(
                            out=cur[bk][1][:, 256 + cc:257 + cc],
                            lhsT=lhsT,
                            rhs=esel[:, bk, 2 * c + 1:2 * c + 2],
                            start=True, stop=True)
            finalize(0)
            finalize(1)

            # ---- tail: local Z0 chain + ln passes (accum_out = lsum),
            # then ONE AllGather of 6 partial sums ----
            for bk in (0, 1):
                zsb = work.tile([128, 1], F32, tag="zsb")
                nc.vector.tensor_reduce(out=zsb[:], in_=redparts[:, bk, :],
                                        axis=mybir.AxisListType.X,
                                        op=mybir.AluOpType.add)
                psz = ps1.tile([1, 1], F32, space="PSUM", tag="red_ps")
                nc.tensor.matmul(out=psz[:], lhsT=ones_col[:], rhs=zsb[:],
                                 start=True, stop=True)
                zi = work.tile([1, 1], F32, tag="zi")
                nc.vector.reciprocal(zi[:], psz[:])
                # zs = 1/(Z0*cp), Z0 = NCORES*zsum_local*N/TOT
                zs = work.tile([1, 1], F32, tag="zs")
                nc.vector.tensor_scalar_mul(
                    zs[:], zi[:], float(TOT) / (NCORES * N_DATA * CP))
                bc_ps = ps1.tile([128, 1], F32, space="PSUM", tag="bc_ps")
                nc.tensor.matmul(out=bc_ps[:], lhsT=ones_row[:], rhs=zs[:],
                                 start=True, stop=True)
                bc_sb = work.tile([128, 1], F32, tag="bcsb")
                nc.vector.tensor_copy(bc_sb[:], bc_ps[:])
                # lsum partial: ln(1 + u*zs) with accum_out; masked u==0 -> 0
                lnscr = work.tile([128, ncht2], F32, tag="lnscr")
                nc.scalar.activation(lnscr[:], ubig[:, bk, :],
                                     mybir.ActivationFunctionType.Ln,
                                     bias=1.0, scale=bc_sb[:],
                                     accum_out=redparts[:, 4 + bk, 0:1])

            red6f = work.tile([128, 6], F32, tag="red6f")
            nc.vector.tensor_reduce(out=red6f[:], in_=redparts[:],
                                    axis=mybir.AxisListType.X,
                                    op=mybir.AluOpType.add)
            psr = ps1.tile([1, 6], F32, space="PSUM", tag="red_ps")
            nc.tensor.matmul(out=psr[:], lhsT=ones_col[:], rhs=red6f[:],
                             start=True, stop=True)
            cc1_sb = work.tile([1, 6], F32, tag="cc1sb")
            nc.vector.tensor_copy(cc1_sb[:], psr[:])
            cc1_in = dram.tile([1, 6], F32, tag="cc1_in")
            nc.sync.dma_start(cc1_in[:], cc1_sb[:])
            nc.gpsimd.collective_compute(
                "AllGather", mybir.AluOpType.bypass,
                replica_groups=[list(range(NCORES))],
                ins=[cc1_in[:]], outs=[cc1_out[:]],
            )
            cc1g8 = work.tile([NCORES, 6], F32, tag="cc1g8")
            nc.sync.dma_start(cc1g8[:], cc1_out[:])
            psg1 = ps1.tile([1, 6], F32, space="PSUM", tag="red_ps")
            nc.tensor.matmul(out=psg1[:], lhsT=ones_col[0:NCORES, :],
                             rhs=cc1g8[:], start=True, stop=True)
            # psg1 = (zsum1, zsum2, pos1, pos2, lsum1, lsum2) globals
            zsc = pers.tile([1, 2], F32, tag="zsc")
            nc.scalar.mul(zsc[:], psg1[:, 0:2], float(N_DATA) / TOT)
            nc.vector.tensor_copy(g8[:, 0:4], psg1[:, 2:6])
            nc.scalar.activation(g8[:, 4:6], zsc[:],
                                 mybir.ActivationFunctionType.Ln)
            # loss = sum(g8 * coef) - c2/B
            w8 = work.tile([1, 6], F32, tag="w8")
            nc.vector.tensor_tensor(out=w8[:], in0=g8[:], in1=coef6[:],
                                    op=mybir.AluOpType.mult)
            r1 = work.tile([1, 1], F32, tag="r1")
            nc.vector.tensor_reduce(out=r1[:], in_=w8[:],
                                    axis=mybir.AxisListType.X,
                                    op=mybir.AluOpType.add)
            loss_sb = work.tile([1, 1], F32, tag="floss")
            nc.vector.tensor_scalar(out=loss_sb[:], in0=r1[:],
                                    scalar1=float(c2 / B), scalar2=None,
                                    op0=mybir.AluOpType.subtract)
            nc.sync.dma_start(out_ext[:], loss_sb[:])

    nc.compile()
    _BUILD_CACHE[key] = nc
    return nc


def run(inputs, trace=False):
    in_maps, nseg, seg, ncht2, ncsel, c2 = _prep_host(inputs)
    nc = _build(nseg, seg, ncht2, ncsel, c2)
    r = run_bass_kernel_spmd(nc, in_maps, list(range(NCORES)), trace=trace)
    loss = np.float32(r.results[0]["out"][0, 0])
    return loss.reshape(()), r


def kernel(**inputs) -> np.ndarray:
    loss, _ = run(inputs)
    return loss


# revision 11
# speedup vs baseline: 2.1698x; 1.0003x over previous
"""CRD (contrastive representation distillation) loss on 8 Trainium2 cores.

Strategy (memory-bound scatter-gather problem):
  - The two 500000x128 memory banks are interleaved row-wise on the host into
    one [500000, 256] bf16 array (m1_row || m2_row), sharded row-wise across
    the 8 cores (62500 rows each, split in two 31250-row halves so gather
    indices fit in int16). One 512-byte dma_gather descriptor then fetches
    BOTH banks' rows for a sampled pair at full DMA-bus efficiency.
  - The host lists, per (core, half), the sampled (b, k) pairs whose memory
    row falls in that shard, in natural b-sorted order with NO per-b padding.
    Each 128-pair chunk spans at most 2 distinct b values (b-runs are ~256
    long; the host packer inserts padding in the astronomically rare case a
    chunk would span 3). Per chunk the host ships the two candidate batch
    ids (as a one-hot matrix) plus a per-pair "hi" mask; padded slots carry
    an additive -2^20 score offset so exp() kills them and ln(1+0)=0.
  - On device, dma_gather(transpose=True) pulls the pair rows from HBM
    already transposed into SBUF tiles [d=128 partitions, 2, pairs]. Each
    128-pair chunk is the stationary matmul operand (lhsT) against the two
    candidate embedding columns; copy_predicated selects hi vs lo.
  - Embeddings es/et (linear head + L2 norm) are computed on every core from
    host-prepacked fp8 f/W tiles (per-tensor scales cancel in the L2 norm).
    Per-chunk embedding columns come from a one-hot matmul (the PE acts as
    the gather), keeping the Pool engine free for the main gather stream.
    1/sqrt(ss) is computed as exp(-0.5*ln(ss)) so every activation lives in
    one table set (no mid-kernel table reloads).
  - Score groups are reduced as soon as their PSUM group fills, overlapped
    with the gather stream (exp's accum_out gives the masked zsum for free).
    The tail is only: last tiny group, the local-Z chain, two ln passes with
    accum_out, one AllGather of 6 partial sums, and a coefficient dot.

The final scalar equals
  loss = (lsum_t - C2 - pos_t/T) / B + logZ1 + logZ2
with C2 = 2*B*K*log(c) - 2*TOT*log(c+eps), c = K/N_DATA, computed on host in
f64 (the naive formulation cancels 2.5e6-magnitude terms, which f32 cannot).
"""

import sys

sys.path.insert(0, "/opt/trn_rl_repo")

import math

import numpy as np
import ml_dtypes

import concourse.bacc as bacc
import concourse.bass as bass
import concourse.mybir as mybir
import concourse.tile as tile
from concourse.bass_utils import run_bass_kernel_spmd

# ---- problem constants (from reference.py) ----
B = 128
K = 4096
KP1 = K + 1
D = 128
N_DATA = 500000
NCE_T = 0.07
EPS = 1e-7
FEAT = 2048

NCORES = 8
RPC = N_DATA // NCORES          # rows per core = 62500
HALF = RPC // 2                 # 31250, fits int16 indexing
TOT = B * KP1                   # 524416 sampled pairs per bank
C_NEG = K / N_DATA              # m * Pn
CP = C_NEG + EPS
PAD_OFF = -1048576.0            # additive score offset for padded slots

F32 = mybir.dt.float32
BF16 = mybir.dt.bfloat16
FP8 = mybir.dt.float8e4
I16 = mybir.dt.int16
NPFP8 = ml_dtypes.float8_e4m3

F_SCALE = 8.0                   # cancels in l2norm; avoids fp8 subnormals
W_SCALE = 32.0

_BUILD_CACHE = {}


def _wrap_idxs(idx_1d: np.ndarray) -> np.ndarray:
    """dma_gather int16 index layout: idx i -> [i % 16, i // 16], replicated
    into all 8 groups of 16 partitions."""
    n = idx_1d.shape[0]
    assert n % 16 == 0
    arr = np.zeros((128, n // 16), np.int16)
    base = idx_1d.reshape(-1, 16).T.astype(np.int16)
    for k in range(8):
        arr[16 * k:16 * k + 16, :] = base
    return arr


def _pack_stream(subs, bs, poss):
    """Pack one (core, half) pair stream: keep natural b-sorted order and pad
    (idx 0, val 0) only when a 128-chunk would otherwise span >2 distinct b.
    Returns per-pair (rows, val, pos, b)."""
    n = len(subs)
    if n == 0:
        return (np.zeros(0, np.int64), np.zeros(0, np.float32),
                np.zeros(0, np.float32), np.zeros(0, np.int64))
    edges = np.flatnonzero(np.concatenate([[1], np.diff(bs) != 0]))
    starts = list(edges) + [n]
    out_r, out_v, out_p, out_b = [], [], [], []
    cur = 0
    last_b = 0
    for ri in range(len(starts) - 1):
        lo, hi = starts[ri], starts[ri + 1]
        bval = int(bs[lo])
        off = cur % 128
        if off:
            c0 = cur - off
            nb = len({int(x) for x in
                      np.concatenate(out_b)[c0:cur]}) if out_b else 0
            if nb >= 2:
                pad = 128 - off
                out_r.append(np.zeros(pad, np.int64))
                out_v.append(np.zeros(pad, np.float32))
                out_p.append(np.zeros(pad, np.float32))
                out_b.append(np.full(pad, last_b, np.int64))
                cur += pad
        cnt = hi - lo
        out_r.append(subs[lo:hi])
        out_v.append(np.ones(cnt, np.float32))
        out_p.append(poss[lo:hi].astype(np.float32))
        out_b.append(np.full(cnt, bval, np.int64))
        cur += cnt
        last_b = bval
    return (np.concatenate(out_r), np.concatenate(out_v),
            np.concatenate(out_p), np.concatenate(out_b))


def _prep_host(inputs):
    f_s = np.asarray(inputs["f_s"], np.float32)
    f_t = np.asarray(inputs["f_t"], np.float32)
    idx = np.asarray(inputs["idx"]).astype(np.int64)
    cidx = np.asarray(inputs["contrast_idx"]).astype(np.int64)
    W_s = np.asarray(inputs["W_s"], np.float32)
    b_s = np.asarray(inputs["b_s"], np.float32)
    W_t = np.asarray(inputs["W_t"], np.float32)
    b_t = np.asarray(inputs["b_t"], np.float32)
    m1 = np.asarray(inputs["memory_v1"], np.float32)
    m2 = np.asarray(inputs["memory_v2"], np.float32)

    full_idx = np.concatenate([idx[:, None], cidx], axis=1)  # [B, KP1]
    b_of = np.broadcast_to(np.arange(B)[:, None], (B, KP1)).ravel()
    k_of = np.broadcast_to(np.arange(KP1)[None, :], (B, KP1)).ravel()
    r_of = full_idx.ravel()
    owner = r_of // RPC
    local = r_of - owner * RPC
    half = local // HALF
    sub = (local % HALF).astype(np.int64)

    per = {}
    maxlen = 0
    for m in range(NCORES):
        for hh in (0, 1):
            sel = (owner == m) & (half == hh)
            packed = _pack_stream(sub[sel], b_of[sel], k_of[sel] == 0)
            per[(m, hh)] = packed
            maxlen = max(maxlen, len(packed[0]))

    # equal segments (mult of 128); the last segment of half 1 is split off
    # small so the final PSUM group finalizes in ~1us
    nsegments = -(-maxlen // 8192)
    seg = -(-maxlen // (nsegments * 128)) * 128
    nseg = nsegments * seg
    nch = nseg // 128
    ncht2 = 2 * nch
    ncsel = -(-2 * ncht2 // 128) * 128

    # prepacked embedding operands: [p, kk, c] = M[kk*128+p, c], fp8 w/ scale
    def pack_pkc(M, scale):
        return ((M * scale).reshape(16, 128, 128).transpose(1, 0, 2)
                .reshape(128, FEAT).astype(NPFP8))

    shared = {
        "fspack": pack_pkc(f_s.T.copy(), F_SCALE),
        "ftpack": pack_pkc(f_t.T.copy(), F_SCALE),
        "wspack": pack_pkc(W_s.copy(), W_SCALE),
        "wtpack": pack_pkc(W_t.copy(), W_SCALE),
        "bias2": (np.stack([b_s, b_t]) * F_SCALE * W_SCALE).astype(NPFP8),
    }

    big = np.concatenate([m1.astype(ml_dtypes.bfloat16),
                          m2.astype(ml_dtypes.bfloat16)], axis=1)  # [N, 256]

    in_maps = []
    for m in range(NCORES):
        imap = dict(shared)
        masks = np.zeros((128, 2 * ncht2), ml_dtypes.bfloat16)
        maskh8 = np.zeros((128, ncht2), np.uint8)
        eselb = np.zeros(ncsel, np.int64)
        onehot = np.zeros((128, ncsel), ml_dtypes.bfloat16)
        idx_parts = []
        for hh in (0, 1):
            rows, val, pos, bofp = per[(m, hh)]
            n = len(rows)
            rows_p = np.zeros(nseg, np.int64)
            rows_p[:n] = rows
            val_p = np.zeros(nseg, np.float32)
            val_p[:n] = val
            pos_p = np.zeros(nseg, np.float32)
            pos_p[:n] = pos
            b_p = np.zeros(nseg, np.int64)
            b_p[:n] = bofp
            idx_parts.append(_wrap_idxs(rows_p))
            co = hh * nch
            madd = np.where(val_p > 0, 0.0, PAD_OFF).astype(np.float32)
            masks[:, co:co + nch] = madd.reshape(nch, 128).T
            masks[:, ncht2 + co:ncht2 + co + nch] = pos_p.reshape(nch, 128).T
            bmat = b_p.reshape(nch, 128)           # [chunk, pair-in-chunk]
            b_lo = bmat[:, 0]
            b_hi = bmat[:, -1]
            hi_mask = ((bmat != b_lo[:, None]) &
                       (val_p.reshape(nch, 128) > 0)).astype(np.uint8)
            maskh8[:, co:co + nch] = hi_mask.T
            cg = np.arange(nch) + co
            eselb[2 * cg] = b_lo
            eselb[2 * cg + 1] = b_hi
            base = m * RPC + hh * HALF
            imap[f"bkh{hh}"] = big[base:base + HALF]
        onehot[eselb, np.arange(ncsel)] = 1.0
        imap["idxall"] = np.concatenate(idx_parts, axis=1)
        imap["masks"] = masks
        imap["maskh8"] = maskh8
        imap["onehot"] = onehot
        in_maps.append(imap)

    c2 = 2.0 * B * K * math.log(C_NEG) - 2.0 * TOT * math.log(CP)
    return in_maps, nseg, seg, ncht2, ncsel, c2


def _build(nseg, seg, ncht2, ncsel, c2):
    key = (nseg, seg, ncht2, ncsel, round(c2, 6))
    if key in _BUILD_CACHE:
        return _BUILD_CACHE[key]
    nch = nseg // 128

    # segment plan: [ (half, chunk0, nchunks) ... ]; split the very last one
    segs = []
    for hh in (0, 1):
        plan = [seg] * (nseg // seg)
        if hh == 1 and seg > 1024:
            plan = plan[:-1] + [seg - 1024, 1024]
        p0 = 0
        for sz in plan:
            segs.append((hh, p0, sz))
            p0 += sz
    last_nch = segs[-1][2] // 128

    # chunk groups (PSUM drains): <=256 chunks, last segment its own group
    cut = ncht2 - last_nch
    bounds = list(range(0, cut, 256)) + [cut, ncht2]
    bounds = sorted(set(bounds))
    NG = len(bounds) - 1
    grp_of = np.zeros(ncht2, np.int64)
    for gi in range(NG):
        grp_of[bounds[gi]:bounds[gi + 1]] = gi

    nc = bacc.Bacc(None, target_bir_lowering=False, debug=False)
    t_in = {}
    for nm in ("fspack", "ftpack", "wspack", "wtpack"):
        t_in[nm] = nc.dram_tensor(nm, [128, FEAT], FP8, kind="ExternalInput")
    bias2_in = nc.dram_tensor("bias2", [2, 128], FP8, kind="ExternalInput")
    banks = {hh: nc.dram_tensor(f"bkh{hh}", [HALF, 2 * D], BF16,
                                kind="ExternalInput")
             for hh in (0, 1)}
    idxall_in = nc.dram_tensor("idxall", [128, 2 * nseg // 16], I16,
                               kind="ExternalInput")
    masks_in = nc.dram_tensor("masks", [128, 2 * ncht2], BF16,
                              kind="ExternalInput")
    maskh_in = nc.dram_tensor("maskh8", [128, ncht2], mybir.dt.uint8,
                              kind="ExternalInput")
    onehot_in = nc.dram_tensor("onehot", [128, ncsel], BF16,
                               kind="ExternalInput")
    out_ext = nc.dram_tensor("out", [1, 1], F32, kind="ExternalOutput")

    cc1_out = nc.dram_tensor("cc1_out_sh", [NCORES, 6], F32, addr_space="Shared")

    with tile.TileContext(nc) as tc:
        with (
            tc.tile_pool(name="persist", bufs=1) as pers,
            tc.tile_pool(name="work", bufs=3) as work,
            tc.tile_pool(name="gather", bufs=3) as gpool,
            tc.tile_pool(name="gather1", bufs=1) as gpool1,
            tc.tile_pool(name="ps", bufs=2, space="PSUM") as ps,
            tc.tile_pool(name="ps1", bufs=1, space="PSUM") as ps1,
            tc.tile_pool(name="dram", bufs=1, space="DRAM") as dram,
        ):
            # ---- load small inputs; idxall first (it gates the gathers) ----
            idxall = pers.tile([128, 2 * nseg // 16], I16, tag="idxall")
            nc.sync.dma_start(idxall[:], idxall_in[:])
            idx_sb = {hh: idxall[:, hh * (nseg // 16):(hh + 1) * (nseg // 16)]
                      for hh in (0, 1)}

            fw_sb = {}
            for nm in ("fspack", "ftpack", "wspack", "wtpack"):
                t = pers.tile([128, FEAT], FP8, tag=nm)
                nc.sync.dma_start(t[:], t_in[nm][:])
                fw_sb[nm] = t
            bias_sb = {}
            for i, which in ((0, "s"), (1, "t")):
                bt = pers.tile([1, 128], FP8, tag=f"bias{which}")
                nc.sync.dma_start(bt[:], bias2_in[i:i + 1, :])
                bias_sb[which] = bt
            onehot_sb = pers.tile([128, ncsel], BF16, tag="onehot")
            nc.sync.dma_start(onehot_sb[:], onehot_in[:])
            masks_bf = pers.tile([128, 2 * ncht2], BF16, tag="masksbf")
            nc.sync.dma_start(masks_bf[:], masks_in[:])
            maskh_sb = pers.tile([128, ncht2], mybir.dt.uint8, tag="maskh")
            nc.sync.dma_start(maskh_sb[:], maskh_in[:])
            masksf = pers.tile([128, 2 * ncht2], F32, tag="masksf")
            nc.vector.tensor_copy(masksf[:], masks_bf[:])
            madd = masksf[:, 0:ncht2]
            maskp = masksf[:, ncht2:2 * ncht2]

            ones_col = pers.tile([128, 1], F32, tag="ones_col")
            nc.vector.memset(ones_col[:], 1.0)
            ones_row8 = pers.tile([1, 128], FP8, tag="ones_row8")
            nc.vector.memset(ones_row8[:], 1.0)
            ones_row = pers.tile([1, 128], F32, tag="ones_row")
            nc.vector.memset(ones_row[:], 1.0)
            redparts = pers.tile([128, 6, NG], F32, tag="redparts")
            nc.vector.memset(redparts[:], 0.0)
            g8 = pers.tile([1, 6], F32, tag="g8")
            nc.vector.memset(g8[:], 0.0)
            coef6 = pers.tile([1, 6], F32, tag="coef6")
            nc.vector.memset(coef6[:, 0:2], -1.0 / (B * NCE_T))
            nc.vector.memset(coef6[:, 2:4], 1.0 / B)
            nc.vector.memset(coef6[:, 4:6], 1.0)

            # ---- embeddings: e = l2norm(f @ W + b), kept in SBUF (bf16);
            # 1/sqrt via exp(-0.5*ln) so only one act table set is used ----
            en = {}
            for which, f_nm, w_nm in (("t", "ftpack", "wtpack"),
                                      ("s", "fspack", "wspack")):
                pe = ps.tile([128, 512], F32, space="PSUM", tag="scratch")
                pev = pe[:, 0:128]
                for kk in range(16):
                    sl = slice(kk * 128, (kk + 1) * 128)
                    nc.tensor.matmul(out=pev, lhsT=fw_sb[f_nm][:, sl],
                                     rhs=fw_sb[w_nm][:, sl],
                                     start=(kk == 0), stop=False)
                nc.tensor.matmul(out=pev, lhsT=ones_row8[:],
                                 rhs=bias_sb[which][:],
                                 start=False, stop=True)
                sq = work.tile([128, 128], F32, tag="emb_sq")
                nc.scalar.square(sq[:], pev)
                ss = work.tile([128, 1], F32, tag="emb_ss")
                nc.vector.tensor_reduce(out=ss[:], in_=sq[:],
                                        axis=mybir.AxisListType.X,
                                        op=mybir.AluOpType.add)
                sr = work.tile([128, 1], F32, tag="emb_sr")
                nc.scalar.sqrt(sr[:], ss[:])
                ri = work.tile([128, 1], F32, tag="emb_ri")
                nc.vector.reciprocal(ri[:], sr[:])
                ent = pers.tile([128, 128], BF16, tag=f"en{which}")
                nc.vector.tensor_scalar_mul(ent[:], pev, ri[:])
                en[which] = ent

            # ---- per-chunk embedding columns via one-hot matmul ----
            # bank 0 = memory_v1 rows scored against e_t; bank 1 = m2 vs e_s
            esel = pers.tile([128, 2, ncsel], BF16, tag="esel")
            for bk, which in ((0, "t"), (1, "s")):
                for blk in range(0, ncsel, 512):
                    bw = min(512, ncsel - blk)
                    pe2 = ps.tile([128, 512], F32, space="PSUM", tag="scratch")
                    nc.tensor.matmul(out=pe2[:, :bw], lhsT=en[which][:],
                                     rhs=onehot_sb[:, blk:blk + bw],
                                     start=True, stop=True)
                    nc.vector.tensor_copy(esel[:, bk, blk:blk + bw],
                                          pe2[:, :bw])

            # ---- main loop: gather interleaved pair rows (512B
            # descriptors), two 1-col matmuls per chunk per bank, and
            # finalize each PSUM group as soon as it fills ----
            selbig = pers.tile([128, 2, ncht2], F32, tag="selbig")
            ubig = pers.tile([128, 2, ncht2], F32, tag="ubig")
            cur = {0: None, 1: None}

            def finalize(bk):
                if cur[bk] is None:
                    return
                g, t = cur[bk]
                lo = bounds[g]
                n = bounds[g + 1] - lo
                sv = selbig[:, bk, lo:lo + n]
                # sel = (pk_lo + pad_offset), overwritten with pk_hi where
                # the hi-mask is set (pads have maskh=0, keeping the offset)
                nc.vector.tensor_tensor(out=sv, in0=t[:, :n],
                                        in1=madd[:, lo:lo + n],
                                        op=mybir.AluOpType.add)
                nc.vector.copy_predicated(sv, maskh_sb[:, lo:lo + n],
                                          t[:, 256:256 + n])
                # u = exp(sel/T); accum_out = masked zsum partial for free
                nc.scalar.activation(ubig[:, bk, lo:lo + n], sv,
                                     mybir.ActivationFunctionType.Exp,
                                     bias=0.0, scale=1.0 / NCE_T,
                                     accum_out=redparts[:, bk, g:g + 1])
                scr = work.tile([128, 256], F32, tag="pscr")
                nc.vector.tensor_tensor(out=scr[:, :n], in0=sv,
                                        in1=maskp[:, lo:lo + n],
                                        op=mybir.AluOpType.mult)
                nc.vector.tensor_reduce(
                    out=redparts[:, 2 + bk, g:g + 1], in_=scr[:, :n],
                    axis=mybir.AxisListType.X, op=mybir.AluOpType.add)
                cur[bk] = None

            for hh, p0, sz in segs:
                pool = gpool if sz == seg else gpool1
                gt = pool.tile([128, 2, sz], BF16, tag=f"gt{sz}",
                               name=f"gt{hh}_{p0}")
                nc.gpsimd.dma_gather(
                    gt[:], banks[hh][:],
                    idx_sb[hh][:, p0 // 16:(p0 + sz) // 16],
                    sz, sz, 2 * D, elem_step=2 * D, transpose=True,
                    single_packet=False,
                )
                for j2 in range(sz // 128):
                    c = hh * nch + p0 // 128 + j2
                    g = int(grp_of[c])
                    cc = c - bounds[g]
                    for bk in (0, 1):
                        if cur[bk] is None or cur[bk][0] != g:
                            finalize(bk)
                            cur[bk] = (g, ps.tile([128, 512], F32,
                                                  space="PSUM",
                                                  tag=f"pk{bk}",
                                                  name=f"pk{bk}_{g}"))
                        lhsT = gt[:, bk, j2 * 128:(j2 + 1) * 128]
                        nc.tensor.matmul(
                            out=cur[bk][1][:, cc:cc + 1], lhsT=lhsT,
                            rhs=esel[:, bk, 2 * c:2 * c + 1],
                            start=True, stop=True)
                        nc.tensor.matmul(
                            out=cur[bk][1][:, 256 + cc:257 + cc],
                            lhsT=lhsT,
                            rhs=esel[:, bk, 2 * c + 1:2 * c + 2],
                            start=True, stop=True)
            finalize(0)
            finalize(1)

            # ---- tail: local Z0 chain + ln passes (accum_out = lsum),
            # then ONE AllGather of 6 partial sums ----
            for bk in (0, 1):
                zsb = work.tile([128, 1], F32, tag="zsb")
                nc.vector.tensor_reduce(out=zsb[:], in_=redparts[:, bk, :],
                                        axis=mybir.AxisListType.X,
                                        op=mybir.AluOpType.add)
                psz = ps1.tile([1, 1], F32, space="PSUM", tag="red_ps")
                nc.tensor.matmul(out=psz[:], lhsT=ones_col[:], rhs=zsb[:],
                                 start=True, stop=True)
                zi = work.tile([1, 1], F32, tag="zi")
                nc.vector.reciprocal(zi[:], psz[:])
                # zs = 1/(Z0*cp), Z0 = NCORES*zsum_local*N/TOT
                zs = work.tile([1, 1], F32, tag="zs")
                nc.vector.tensor_scalar_mul(
                    zs[:], zi[:], float(TOT) / (NCORES * N_DATA * CP))
                bc_ps = ps1.tile([128, 1], F32, space="PSUM", tag="bc_ps")
                nc.tensor.matmul(out=bc_ps[:], lhsT=ones_row[:], rhs=zs[:],
                                 start=True, stop=True)
                bc_sb = work.tile([128, 1], F32, tag="bcsb")
                nc.vector.tensor_copy(bc_sb[:], bc_ps[:])
                # lsum partial: ln(1 + u*zs) with accum_out; masked u==0 -> 0
                lnscr = work.tile([128, ncht2], F32, tag="lnscr")
                nc.scalar.activation(lnscr[:], ubig[:, bk, :],
                                     mybir.ActivationFunctionType.Ln,
                                     bias=1.0, scale=bc_sb[:],
                                     accum_out=redparts[:, 4 + bk, 0:1])

            red6f = work.tile([128, 6], F32, tag="red6f")
            nc.vector.tensor_reduce(out=red6f[:], in_=redparts[:],
                                    axis=mybir.AxisListType.X,
                                    op=mybir.AluOpType.add)
            psr = ps1.tile([1, 6], F32, space="PSUM", tag="red_ps")
            nc.tensor.matmul(out=psr[:], lhsT=ones_col[:], rhs=red6f[:],
                             start=True, stop=True)
            cc1_sb = work.tile([1, 6], F32, tag="cc1sb")
            nc.vector.tensor_copy(cc1_sb[:], psr[:])
            cc1_in = dram.tile([1, 6], F32, tag="cc1_in")
            nc.sync.dma_start(cc1_in[:], cc1_sb[:])
            nc.gpsimd.collective_compute(
                "AllGather", mybir.AluOpType.bypass,
                replica_groups=[list(range(NCORES))],
                ins=[cc1_in[:]], outs=[cc1_out[:]],
            )
            cc1g8 = work.tile([NCORES, 6], F32, tag="cc1g8")
            nc.sync.dma_start(cc1g8[:], cc1_out[:])
            psg1 = ps1.tile([1, 6], F32, space="PSUM", tag="red_ps")
            nc.tensor.matmul(out=psg1[:], lhsT=ones_col[0:NCORES, :],
                             rhs=cc1g8[:], start=True, stop=True)
            # psg1 = (zsum1, zsum2, pos1, pos2, lsum1, lsum2) globals
            nc.vector.tensor_copy(g8[:, 0:4], psg1[:, 2:6])
            nc.scalar.activation(g8[:, 4:6], psg1[:, 0:2],
                                 mybir.ActivationFunctionType.Ln,
                                 bias=0.0, scale=float(N_DATA) / TOT)
            # loss = sum(g8 * coef) - c2/B
            w8 = work.tile([1, 6], F32, tag="w8")
            nc.vector.tensor_tensor(out=w8[:], in0=g8[:], in1=coef6[:],
                                    op=mybir.AluOpType.mult)
            r1 = work.tile([1, 1], F32, tag="r1")
            nc.vector.tensor_reduce(out=r1[:], in_=w8[:],
                                    axis=mybir.AxisListType.X,
                                    op=mybir.AluOpType.add)
            loss_sb = work.tile([1, 1], F32, tag="floss")
            nc.vector.tensor_scalar(out=loss_sb[:], in0=r1[:],
                                    scalar1=float(c2 / B), scalar2=None,
                                    op0=mybir.AluOpType.subtract)
            nc.sync.dma_start(out_ext[:], loss_sb[:])

    nc.compile()
    _BUILD_CACHE[key] = nc
    return nc


def run(inputs, trace=False):
    in_maps, nseg, seg, ncht2, ncsel, c2 = _prep_host(inputs)
    nc = _build(nseg, seg, ncht2, ncsel, c2)
    r = run_bass_kernel_spmd(nc, in_maps, list(range(NCORES)), trace=trace)
    loss = np.float32(r.results[0]["out"][0, 0])
    return loss.reshape(()), r


def kernel(**inputs) -> np.ndarray:
    loss, _ = run(inputs)
    return loss
